# revision 3
# baseline (speedup 1.0000x reference)
"""Causal self-attention (RoPE on k/v) TRN2 Bass kernel.

Sharding: core i handles batch b = i//2 and 8 heads hs = 8*(i%2).
Each core computes qkv projection for its (batch, head-group), RoPE on
k and v, causal attention, and a partial output projection y^T with its
W_proj row-block.  Host sums the two partials per batch and adds b_proj.

Layouts on device (per core):
  xT   [C, T]     x[b]^T (host-transposed)
  qkT  [2048, T]  rows 0-1023 = q^T (head-major, perm'd d), 1024-2047 = rope(k)^T
  vr   [T, 1024]  rope(v), natural layout
  oT   [1024, T]  attention out^T
  yT   [C, T]     partial out-proj (output)

Head-dim permutation (even dims first) turns RoPE's interleaved
even/odd pairs into contiguous 64-row/col halves; W_attn columns and
W_proj rows are permuted correspondingly on host, which leaves the
attention math invariant.

All matmul operands are float32r (fp32 bits, 1 cycle/row on PE at
N>=256 vs 4 for plain fp32; measured relerr 1.5e-4 vs fp64, identical
to the fp32 matmul path on this HW).
"""
import sys

sys.path.insert(0, "/opt/trn_rl_repo")

import numpy as np

import concourse.bass as bass  # noqa: F401
import concourse.mybir as mybir
import concourse.tile as tile
from concourse import bacc
from concourse.bass_utils import run_bass_kernel_spmd

B, T, C, H = 4, 2048, 2048, 16
HD = 128
HC = 8  # heads per core
NCORES = 8
F32 = mybir.dt.float32
F32R = mybir.dt.float32r
SCALE = float(1.0 / np.sqrt(HD))

_CACHE = {}


def _build_nc():
    nc = bacc.Bacc(num_devices=NCORES)

    xT = nc.dram_tensor("xT", [C, T], F32R, kind="ExternalInput")
    wqk = nc.dram_tensor("wqk", [C, 2048], F32R, kind="ExternalInput")
    bqk = nc.dram_tensor("bqk", [128, 16], F32, kind="ExternalInput")
    wv = nc.dram_tensor("wv", [C, 1024], F32R, kind="ExternalInput")
    bv = nc.dram_tensor("bv", [128, 1024], F32, kind="ExternalInput")
    wp = nc.dram_tensor("wp", [1024, C], F32R, kind="ExternalInput")
    rtab_u = nc.dram_tensor("rtab_u", [128, T], F32, kind="ExternalInput")
    rtab_v = nc.dram_tensor("rtab_v", [128, T], F32, kind="ExternalInput")
    cos4 = nc.dram_tensor("cos4", [T, 256], F32, kind="ExternalInput")
    sin4 = nc.dram_tensor("sin4", [T, 256], F32, kind="ExternalInput")
    masks = nc.dram_tensor("masks", [128, 2048], F32, kind="ExternalInput")
    yT = nc.dram_tensor("yT", [C, T], F32, kind="ExternalOutput")

    qkT = nc.dram_tensor("qkT", [2048, T], F32R)
    vr_d = nc.dram_tensor("vr_d", [T, 1024], F32R)
    oT_d = nc.dram_tensor("oT_d", [1024, T], F32R)

    with tile.TileContext(nc) as tc:
        if True:
            # ---------------- Phase A1: q^T and rope(k)^T ----------------
            # xT resident (128KB/partition); W_qk streamed per column tile.
            with tc.tile_pool(name="xt", bufs=1) as xtp, \
                 tc.tile_pool(name="a1tab", bufs=1) as atabp, \
                 tc.tile_pool(name="wblk", bufs=2) as wbp, \
                 tc.tile_pool(name="qko", bufs=4) as qkop, \
                 tc.tile_pool(name="ktmp", bufs=2) as ktp, \
                 tc.tile_pool(name="psA1", bufs=6, space="PSUM") as psp:
                xt = xtp.tile([128, 16, T], F32R)
                nc.sync.dma_start(xt[:], xT.rearrange("(a p) t -> p a t", p=128))
                ut = atabp.tile([128, T], F32)
                nc.sync.dma_start(ut[:], rtab_u[:, :])
                vt_tab = atabp.tile([128, T], F32)
                nc.sync.dma_start(vt_tab[:], rtab_v[:, :])
                bqk_t = atabp.tile([128, 16], F32)
                nc.sync.dma_start(bqk_t[:], bqk[:, :])

                wqk_r = wqk.rearrange("(a p) j -> p a j", p=128)
                for jt in range(16):
                    wblk = wbp.tile([128, 16, 128], F32R, tag="wblk")
                    nc.sync.dma_start(
                        wblk[:], wqk_r[:, :, jt * 128:(jt + 1) * 128])
                    for tb in range(4):
                        ts = bass.ts(tb, 512)
                        ps = psp.tile([128, 512], F32, tag="ps")
                        for c in range(16):
                            nc.tensor.matmul(
                                ps[:], wblk[:, c], xt[:, c, ts],
                                start=(c == 0), stop=(c == 15))
                        if jt < 8:
                            qo = qkop.tile([128, 512], F32R, tag="qko")
                            nc.vector.tensor_scalar_add(
                                qo[:], ps[:], bqk_t[:, jt:jt + 1])
                            nc.sync.dma_start(
                                qkT[jt * 128:(jt + 1) * 128, ts], qo[:])
                        else:
                            kt = ktp.tile([128, 512], F32, tag="kt")
                            nc.vector.tensor_scalar_add(
                                kt[:], ps[:], bqk_t[:, jt:jt + 1])
                            kts = ktp.tile([128, 512], F32, tag="kts")
                            nc.sync.dma_start(kts[0:64, :], kt[64:128, :])
                            nc.sync.dma_start(kts[64:128, :], kt[0:64, :])
                            m1 = ktp.tile([128, 512], F32, tag="m1")
                            nc.vector.tensor_mul(m1[:], kt[:], ut[:, ts])
                            m2 = ktp.tile([128, 512], F32, tag="m2")
                            nc.vector.tensor_mul(
                                m2[:], kts[:], vt_tab[:, ts])
                            ko = qkop.tile([128, 512], F32R, tag="qko")
                            nc.vector.tensor_add(ko[:], m1[:], m2[:])
                            nc.sync.dma_start(
                                qkT[jt * 128:(jt + 1) * 128, ts], ko[:])

            # ---------------- Phase A2: v natural + rope ----------------
            # Full Wv resident (64KB/partition); xT tiles streamed.
            with tc.tile_pool(name="wvf", bufs=1) as wvp, \
                 tc.tile_pool(name="xa", bufs=3) as xap, \
                 tc.tile_pool(name="vtab", bufs=1) as vtabp, \
                 tc.tile_pool(name="vro", bufs=4) as vrop, \
                 tc.tile_pool(name="vtmp", bufs=2) as vtp, \
                 tc.tile_pool(name="psA2", bufs=6, space="PSUM") as psp:
                wvf = wvp.tile([128, 16, 1024], F32R)
                nc.sync.dma_start(wvf[:], wv.rearrange("(a p) d -> p a d", p=128))
                bv_t = vtabp.tile([128, 1024], F32)
                nc.sync.dma_start(bv_t[:], bv[:, :])
                c4 = vtabp.tile([128, 16, 256], F32)
                nc.sync.dma_start(
                    c4[:], cos4.rearrange("(a p) i -> p a i", p=128))
                s4 = vtabp.tile([128, 16, 256], F32)
                nc.sync.dma_start(
                    s4[:], sin4.rearrange("(a p) i -> p a i", p=128))

                xT_r = xT.rearrange("(a p) t -> p a t", p=128)
                for tt in range(16):
                    xa = xap.tile([128, 16, 128], F32R, tag="xa")
                    nc.sync.dma_start(
                        xa[:], xT_r[:, :, bass.ts(tt, 128)])
                    for db in range(2):
                        ds = bass.ts(db, 512)
                        ps = psp.tile([128, 512], F32, tag="ps")
                        for c in range(16):
                            nc.tensor.matmul(
                                ps[:], xa[:, c],
                                wvf[:, c, ds], start=(c == 0), stop=(c == 15))
                        vtmp = vtp.tile([128, 512], F32, tag="vtmp")
                        nc.vector.tensor_add(vtmp[:], ps[:], bv_t[:, ds])
                        v3 = vtmp[:].rearrange("p (h d) -> p h d", h=4)
                        c43 = c4[:, tt].rearrange("p (h d) -> p h d", h=4)
                        s43 = s4[:, tt].rearrange("p (h d) -> p h d", h=4)
                        me = vtp.tile([128, 4, 64], F32, tag="me")
                        mo = vtp.tile([128, 4, 64], F32, tag="mo")
                        vro = vrop.tile([128, 512], F32R, tag="vro")
                        vr3 = vro[:].rearrange("p (h d) -> p h d", h=4)
                        nc.vector.tensor_mul(
                            me[:], v3[:, :, 0:64], c43[:, :, 0:64])
                        nc.vector.tensor_mul(
                            mo[:], v3[:, :, 64:128], s43[:, :, 0:64])
                        nc.vector.tensor_sub(
                            vr3[:, :, 0:64], me[:], mo[:])
                        nc.vector.tensor_mul(
                            me[:], v3[:, :, 0:64], s43[:, :, 0:64])
                        nc.vector.tensor_mul(
                            mo[:], v3[:, :, 64:128], c43[:, :, 0:64])
                        nc.vector.tensor_add(
                            vr3[:, :, 64:128], me[:], mo[:])
                        nc.sync.dma_start(
                            vr_d[bass.ts(tt, 128), ds], vro[:])

            # ---------------- Phase B: attention per head ----------------
            with tc.tile_pool(name="hk", bufs=2) as hkp, \
                 tc.tile_pool(name="hq", bufs=2) as hqp, \
                 tc.tile_pool(name="hv", bufs=2) as hvp, \
                 tc.tile_pool(name="pt", bufs=4) as ptp, \
                 tc.tile_pool(name="bsc", bufs=2) as bscp, \
                 tc.tile_pool(name="oo", bufs=2) as oop, \
                 tc.tile_pool(name="const", bufs=1) as constp, \
                 tc.tile_pool(name="psB", bufs=5, space="PSUM") as psp, \
                 tc.tile_pool(name="lps", bufs=1, space="PSUM") as lpsp, \
                 tc.tile_pool(name="ops", bufs=2, space="PSUM") as opsp:
                ones_f = constp.tile([128, 1], F32)
                nc.vector.memset(ones_f[:], 1.0)
                ones_t = constp.tile([128, 1], F32R)
                nc.vector.tensor_copy(ones_t[:], ones_f[:])
                masks_t = constp.tile([128, 4, 512], F32)
                nc.sync.dma_start(
                    masks_t[:], masks.rearrange("p (r i) -> p r i", r=4))
                vr_r = vr_d.rearrange("(jt p) d -> p jt d", p=128)
                for h in range(HC):
                    krh = hkp.tile([128, T], F32R, tag="krh")
                    nc.sync.dma_start(
                        krh[:], qkT[1024 + h * 128:1024 + (h + 1) * 128, :])
                    qh = hqp.tile([128, T], F32R, tag="qh")
                    nc.sync.dma_start(qh[:], qkT[h * 128:(h + 1) * 128, :])
                    vh = hvp.tile([128, 16, 128], F32R, tag="vh")
                    nc.sync.dma_start(
                        vh[:], vr_r[:, :, h * 128:(h + 1) * 128])

                    for ib in range(4):
                        isl = bass.ts(ib, 512)
                        nj = 4 * ib + 4
                        l_ps = lpsp.tile([1, 512], F32, tag="l")
                        o_ps = opsp.tile([128, 512], F32, tag="o")
                        pts = [None] * nj

                        def consume(jt):
                            pt = pts[jt]
                            nc.tensor.matmul(
                                l_ps[:], ones_t[:], pt[:],
                                start=(jt == 0), stop=(jt == nj - 1))
                            nc.tensor.matmul(
                                o_ps[:], vh[:, jt], pt[:],
                                start=(jt == 0), stop=(jt == nj - 1))

                        for jt in range(nj):
                            s_ps = psp.tile([128, 512], F32, tag="ps")
                            nc.tensor.matmul(
                                s_ps[:], krh[:, bass.ts(jt, 128)],
                                qh[:, isl], start=True, stop=True)
                            pt = ptp.tile([128, 512], F32R, tag="pt")
                            nc.scalar.activation(
                                pt[:], s_ps[:],
                                mybir.ActivationFunctionType.Exp, scale=SCALE)
                            if jt >= 4 * ib:
                                nc.vector.tensor_mul(
                                    pt[:], pt[:], masks_t[:, jt - 4 * ib])
                            pts[jt] = pt
                            if jt >= 1:
                                consume(jt - 1)
                        consume(nj - 1)

                        r_sb = bscp.tile([1, 512], F32, tag="r")
                        nc.vector.reciprocal(r_sb[:], l_ps[:])
                        rb = bscp.tile([128, 512], F32, tag="rb")
                        nc.gpsimd.partition_broadcast(rb[:], r_sb[:])
                        oo = oop.tile([128, 512], F32R, tag="oo")
                        nc.vector.tensor_mul(oo[:], o_ps[:], rb[:])
                        nc.sync.dma_start(
                            oT_d[h * 128:(h + 1) * 128, isl], oo[:])

            # ---------------- Phase C: out projection ----------------
            with tc.tile_pool(name="wpb", bufs=1) as wpp, \
                 tc.tile_pool(name="otb", bufs=2) as otbp, \
                 tc.tile_pool(name="yo", bufs=4) as yop, \
                 tc.tile_pool(name="psC", bufs=6, space="PSUM") as psp:
                wps = wpp.tile([128, 8, C], F32R)
                nc.sync.dma_start(
                    wps[:], wp.rearrange("(ht p) c -> p ht c", p=128))
                oT_r = oT_d.rearrange("(ht p) t -> p ht t", p=128)
                for tb in range(4):
                    ts = bass.ts(tb, 512)
                    otb = otbp.tile([128, 8, 512], F32R, tag="otb")
                    nc.sync.dma_start(otb[:], oT_r[:, :, ts])
                    for ct in range(16):
                        ps = psp.tile([128, 512], F32, tag="ps")
                        for ht in range(8):
                            nc.tensor.matmul(
                                ps[:], wps[:, ht, bass.ts(ct, 128)],
                                otb[:, ht], start=(ht == 0), stop=(ht == 7))
                        yo = yop.tile([128, 512], F32, tag="yo")
                        nc.vector.tensor_copy(yo[:], ps[:])
                        nc.sync.dma_start(
                            yT[ct * 128:(ct + 1) * 128, ts], yo[:])

    nc.compile()
    return nc


def _prep_inputs(x, freqs_cos, freqs_sin, W_attn, b_attn, W_proj):
    """Host-side sharding / layout prep.  Returns list of 8 in_maps."""
    perm = np.concatenate([np.arange(0, HD, 2), np.arange(1, HD, 2)])

    cosT = np.ascontiguousarray(freqs_cos.T)  # [64, T]
    sinT = np.ascontiguousarray(freqs_sin.T)
    rtab_u = np.concatenate([cosT, cosT], axis=0).astype(np.float32)
    rtab_v = np.concatenate([-sinT, sinT], axis=0).astype(np.float32)
    cos4 = np.tile(freqs_cos, (1, 4)).astype(np.float32)  # [T, 256]
    sin4 = np.tile(freqs_sin, (1, 4)).astype(np.float32)

    jj = np.arange(128)[:, None]
    ii = np.arange(512)[None, :]
    masks = np.concatenate(
        [((r * 128 + jj) <= ii).astype(np.float32) for r in range(4)],
        axis=1)  # [128, 2048]

    in_maps = []
    for core in range(NCORES):
        b = core // 2
        hs = HC * (core % 2)
        cols = np.concatenate(
            [g * HD + perm for g in range(hs, hs + HC)])  # [1024]

        wqk = np.concatenate(
            [W_attn[:, cols], W_attn[:, C + cols]], axis=1)
        bqk_flat = np.concatenate([b_attn[cols], b_attn[C + cols]])
        bqk = np.ascontiguousarray(
            bqk_flat.reshape(16, 128).T)  # [128, 16], bias[jt*128+p]
        wv = W_attn[:, 2 * C + cols]
        bv = np.broadcast_to(b_attn[2 * C + cols], (128, 1024))
        wp = W_proj[cols, :]

        in_maps.append({
            "xT": np.ascontiguousarray(x[b].T).astype(np.float32),
            "wqk": np.ascontiguousarray(wqk).astype(np.float32),
            "bqk": np.ascontiguousarray(bqk).astype(np.float32),
            "wv": np.ascontiguousarray(wv).astype(np.float32),
            "bv": np.ascontiguousarray(bv).astype(np.float32),
            "wp": np.ascontiguousarray(wp).astype(np.float32),
            "rtab_u": rtab_u,
            "rtab_v": rtab_v,
            "cos4": cos4,
            "sin4": sin4,
            "masks": np.ascontiguousarray(masks),
        })
    return in_maps


def kernel(x, freqs_cos, freqs_sin, mask, W_attn, b_attn, W_proj, b_proj,
           _return_results=False, _trace=False):
    x = np.asarray(x, dtype=np.float32)
    freqs_cos = np.asarray(freqs_cos, dtype=np.float32)
    freqs_sin = np.asarray(freqs_sin, dtype=np.float32)
    W_attn = np.asarray(W_attn, dtype=np.float32)
    b_attn = np.asarray(b_attn, dtype=np.float32)
    W_proj = np.asarray(W_proj, dtype=np.float32)
    b_proj = np.asarray(b_proj, dtype=np.float32)

    if "nc" not in _CACHE:
        _CACHE["nc"] = _build_nc()
    nc = _CACHE["nc"]

    in_maps = _prep_inputs(x, freqs_cos, freqs_sin, W_attn, b_attn, W_proj)
    res = run_bass_kernel_spmd(nc, in_maps, core_ids=list(range(NCORES)),
                               trace=_trace)

    out = np.empty((B, T, C), dtype=np.float32)
    for b in range(B):
        yt0 = res.results[2 * b]["yT"]
        yt1 = res.results[2 * b + 1]["yT"]
        out[b] = yt0.T + yt1.T + b_proj[None, :]
    if _return_results:
        return out, res
    return out



# revision 11
# speedup vs baseline: 1.1242x; 1.1242x over previous
"""Causal self-attention (RoPE on k/v) TRN2 Bass kernel — fused pipeline.

Sharding: core i handles batch b = i//2 and 8 heads hs = 8*(i%2).

Single fused device program per core, all matmul operands bf16 (fp32
PSUM accumulate; fp8 fails the max-error gate because early causal rows
expose elementwise value/probability quantization directly):

  A (x2 T-halves, resident x^T half = 32KB/part):
    A1: q^T, rope(k)^T projections.  k^T -> k_sb resident SBUF bf16
        [128, 8, T]; q^T -> qT_d DRAM round-trip (cheap, hidden).
    A2: v projection + RoPE(v) -> v_sb resident bf16 [128, 16, 1024].
  B:  attention, ib(q-block)-outer / head-inner: scores bf16, exp on
      Scalar -> pt bf16, l (softmax denom) via ones-matmul, 1/l via
      exp(-ln(l)) on Scalar, normalized o tiles (bf16) stay in SBUF.
  C:  out-projection partial y^T per 512-col block interleaved after
      each ib completes, reading oo tiles from SBUF.  Host sums the two
      partials per batch and adds b_proj.

Head-dim permutation (even dims first) turns RoPE's interleaved
even/odd pairs into contiguous halves; W_attn columns and W_proj rows
are permuted correspondingly on host, which leaves the math invariant.
"""
import sys

sys.path.insert(0, "/opt/trn_rl_repo")

import numpy as np

import concourse.bass as bass  # noqa: F401
import concourse.mybir as mybir
import concourse.tile as tile
from concourse import bacc
from concourse.bass_utils import run_bass_kernel_spmd

B, T, C, H = 4, 2048, 2048, 16
HD = 128
HC = 8  # heads per core
NCORES = 8
F32 = mybir.dt.float32
BF16 = mybir.dt.bfloat16
SCALE = float(1.0 / np.sqrt(HD))
IDENT = mybir.ActivationFunctionType.Identity
EXPF = mybir.ActivationFunctionType.Exp
LNF = mybir.ActivationFunctionType.Ln

_CACHE = {}


def _build_nc():
    nc = bacc.Bacc(num_devices=NCORES)

    xT = nc.dram_tensor("xT", [C, T], BF16, kind="ExternalInput")
    wqk = nc.dram_tensor("wqk", [C, 2048], BF16, kind="ExternalInput")
    bqk = nc.dram_tensor("bqk", [128, 16], F32, kind="ExternalInput")
    wv = nc.dram_tensor("wv", [C, 1024], BF16, kind="ExternalInput")
    bv = nc.dram_tensor("bv", [128, 1024], F32, kind="ExternalInput")
    wp = nc.dram_tensor("wp", [1024, C], BF16, kind="ExternalInput")
    rtab_u = nc.dram_tensor("rtab_u", [128, T], BF16, kind="ExternalInput")
    rtab_v = nc.dram_tensor("rtab_v", [128, T], BF16, kind="ExternalInput")
    cos4 = nc.dram_tensor("cos4", [T, 256], BF16, kind="ExternalInput")
    sin4 = nc.dram_tensor("sin4", [T, 256], BF16, kind="ExternalInput")
    masks = nc.dram_tensor("masks", [128, 2048], BF16, kind="ExternalInput")
    ones_in = nc.dram_tensor("ones_in", [128, 1], BF16, kind="ExternalInput")
    yT = nc.dram_tensor("yT", [C, T], F32, kind="ExternalOutput")

    qT_d = nc.dram_tensor("qT_d", [1024, T], BF16)

    with tile.TileContext(nc) as tc:
        with tc.tile_pool(name="persist", bufs=1) as pp:
            # cross-phase SBUF-resident tensors
            k_sb = pp.tile([128, 8, T], BF16)        # 32KB/part
            v_sb = pp.tile([128, 16, 1024], BF16)    # 32KB/part

            # ---------------- Phase A (two T-halves) ----------------
            with tc.tile_pool(name="xtp", bufs=1) as xtp, \
                 tc.tile_pool(name="a1tab", bufs=1) as atabp, \
                 tc.tile_pool(name="a2tab", bufs=1) as a2tabp, \
                 tc.tile_pool(name="wblk", bufs=2) as wbp, \
                 tc.tile_pool(name="qo", bufs=3) as qop, \
                 tc.tile_pool(name="ktmp", bufs=2) as ktp, \
                 tc.tile_pool(name="vtmp", bufs=2) as vtp, \
                 tc.tile_pool(name="psA", bufs=5, space="PSUM") as psp:
                ut = atabp.tile([128, T], BF16)
                nc.sync.dma_start(ut[:], rtab_u[:, :])
                vt_tab = atabp.tile([128, T], BF16)
                nc.sync.dma_start(vt_tab[:], rtab_v[:, :])
                bqk_t = atabp.tile([128, 16], F32)
                nc.sync.dma_start(bqk_t[:], bqk[:, :])

                # A2 weights/tables prefetched during first A1
                wvf = a2tabp.tile([128, 16, 1024], BF16)
                nc.sync.dma_start(wvf[:], wv.rearrange("(a p) d -> p a d", p=128))
                bv_t = a2tabp.tile([128, 1024], F32)
                nc.sync.dma_start(bv_t[:], bv[:, :])
                c4 = a2tabp.tile([128, 16, 256], BF16)
                nc.sync.dma_start(c4[:], cos4.rearrange("(a p) i -> p a i", p=128))
                s4 = a2tabp.tile([128, 16, 256], BF16)
                nc.sync.dma_start(s4[:], sin4.rearrange("(a p) i -> p a i", p=128))

                xT_r = xT.rearrange("(a p) t -> p a t", p=128)
                wqk_r = wqk.rearrange("(a p) j -> p a j", p=128)
                for hf in range(2):
                    hoff = hf * 1024
                    # resident x^T half (bf16), 2 column chunks
                    xt = xtp.tile([128, 16, 1024], BF16, tag="xt")
                    for tb in range(2):
                        ts = bass.ts(tb, 512)
                        nc.sync.dma_start(
                            xt[:, :, ts],
                            xT_r[:, :, hoff + tb * 512:hoff + (tb + 1) * 512])

                    # ---- A1: q^T -> DRAM, rope(k)^T -> k_sb ----
                    for jt in range(16):
                        wblk = wbp.tile([128, 16, 128], BF16, tag="wblk")
                        nc.sync.dma_start(
                            wblk[:], wqk_r[:, :, jt * 128:(jt + 1) * 128])
                        for tb in range(2):
                            ts = bass.ts(tb, 512)
                            gts = slice(hoff + tb * 512, hoff + (tb + 1) * 512)
                            ps = psp.tile([128, 512], F32, tag="ps")
                            for c in range(16):
                                nc.tensor.matmul(
                                    ps[:], wblk[:, c], xt[:, c, ts],
                                    start=(c == 0), stop=(c == 15))
                            if jt < 8:
                                # q: bias add on Scalar, bf16, to DRAM
                                qo = qop.tile([128, 512], BF16, tag="qo")
                                nc.scalar.activation(
                                    qo[:], ps[:], IDENT,
                                    bias=bqk_t[:, jt:jt + 1])
                                nc.sync.dma_start(
                                    qT_d[jt * 128:(jt + 1) * 128, gts], qo[:])
                            else:
                                kt = ktp.tile([128, 512], F32, tag="kt")
                                nc.scalar.activation(
                                    kt[:], ps[:], IDENT,
                                    bias=bqk_t[:, jt:jt + 1])
                                kts = ktp.tile([128, 512], F32, tag="kts")
                                nc.sync.dma_start(kts[0:64, :], kt[64:128, :])
                                nc.sync.dma_start(kts[64:128, :], kt[0:64, :])
                                m1 = ktp.tile([128, 512], F32, tag="m1")
                                nc.vector.tensor_mul(
                                    m1[:], kt[:], ut[:, gts])
                                m2 = ktp.tile([128, 512], F32, tag="m2")
                                nc.vector.tensor_mul(
                                    m2[:], kts[:], vt_tab[:, gts])
                                nc.vector.tensor_add(
                                    k_sb[:, jt - 8, gts], m1[:], m2[:])

                    # ---- A2: v natural + rope into v_sb ----
                    for tt in range(8):
                        tts = bass.ts(tt, 128)
                        gtt = hf * 8 + tt
                        for db in range(2):
                            ds = bass.ts(db, 512)
                            ps = psp.tile([128, 512], F32, tag="ps")
                            for c in range(16):
                                nc.tensor.matmul(
                                    ps[:], xt[:, c, tts], wvf[:, c, ds],
                                    start=(c == 0), stop=(c == 15))
                            vtmp = vtp.tile([128, 512], F32, tag="vtmp")
                            nc.vector.tensor_add(vtmp[:], ps[:], bv_t[:, ds])
                            v3 = vtmp[:].rearrange("p (h d) -> p h d", h=4)
                            c43 = c4[:, gtt].rearrange("p (h d) -> p h d", h=4)
                            s43 = s4[:, gtt].rearrange("p (h d) -> p h d", h=4)
                            me = vtp.tile([128, 4, 64], F32, tag="me")
                            mo = vtp.tile([128, 4, 64], F32, tag="mo")
                            vr3 = v_sb[:, gtt, ds].rearrange(
                                "p (h d) -> p h d", h=4)
                            nc.vector.tensor_mul(
                                me[:], v3[:, :, 0:64], c43[:, :, 0:64])
                            nc.vector.tensor_mul(
                                mo[:], v3[:, :, 64:128], s43[:, :, 0:64])
                            nc.vector.tensor_sub(
                                vr3[:, :, 0:64], me[:], mo[:])
                            nc.vector.tensor_mul(
                                me[:], v3[:, :, 0:64], s43[:, :, 0:64])
                            nc.vector.tensor_mul(
                                mo[:], v3[:, :, 64:128], c43[:, :, 0:64])
                            nc.vector.tensor_add(
                                vr3[:, :, 64:128], me[:], mo[:])

            # ---------------- Phase B + C fused ----------------
            with tc.tile_pool(name="bconst", bufs=1) as constp, \
                 tc.tile_pool(name="wpb", bufs=1) as wpp, \
                 tc.tile_pool(name="qh", bufs=4) as qhp, \
                 tc.tile_pool(name="ptp", bufs=6) as ptp, \
                 tc.tile_pool(name="bsc", bufs=3) as bscp, \
                 tc.tile_pool(name="oo", bufs=10) as oop, \
                 tc.tile_pool(name="yo", bufs=3) as yop, \
                 tc.tile_pool(name="psS", bufs=3, space="PSUM") as pssp, \
                 tc.tile_pool(name="psO", bufs=2, space="PSUM") as psop, \
                 tc.tile_pool(name="psL", bufs=1, space="PSUM") as pslp, \
                 tc.tile_pool(name="psC", bufs=2, space="PSUM") as pscp:
                masks_t = constp.tile([128, 4, 512], BF16)
                nc.sync.dma_start(
                    masks_t[:], masks.rearrange("p (r i) -> p r i", r=4))
                ones_t = constp.tile([128, 1], BF16)
                nc.sync.dma_start(ones_t[:], ones_in[:, :])
                wps = wpp.tile([128, 8, C], BF16)
                nc.sync.dma_start(
                    wps[:], wp.rearrange("(ht p) c -> p ht c", p=128))

                for ib in range(4):
                    isl = bass.ts(ib, 512)
                    nj = 4 * (ib + 1)
                    oo_tiles = []
                    for h in range(HC):
                        qh = qhp.tile([128, 512], BF16, tag="qh")
                        nc.sync.dma_start(
                            qh[:], qT_d[h * 128:(h + 1) * 128, isl])

                        pend = []

                        def consume(o_ps, l_ps, pt, jt, h):
                            nc.tensor.matmul(
                                l_ps[:], ones_t[:], pt[:],
                                start=(jt == 0), stop=(jt == nj - 1))
                            nc.tensor.matmul(
                                o_ps[:],
                                v_sb[:, jt, h * 128:(h + 1) * 128], pt[:],
                                start=(jt == 0), stop=(jt == nj - 1))

                        l_ps = pslp.tile([1, 512], F32, tag="l")
                        o_ps = psop.tile([128, 512], F32, tag="o")
                        for jt in range(nj):
                            s_ps = pssp.tile([128, 512], F32, tag="s")
                            nc.tensor.matmul(
                                s_ps[:],
                                k_sb[:, h, jt * 128:(jt + 1) * 128],
                                qh[:], start=True, stop=True)
                            pt = ptp.tile([128, 512], BF16, tag="pt")
                            nc.scalar.activation(
                                pt[:], s_ps[:], EXPF, scale=SCALE)
                            if jt >= 4 * ib:
                                nc.vector.tensor_mul(
                                    pt[:], pt[:], masks_t[:, jt - 4 * ib])
                            pend.append((o_ps, l_ps, pt, jt, h))
                            if len(pend) > 3:
                                consume(*pend.pop(0))
                        for args in pend:
                            consume(*args)

                        # normalize: r = exp(-ln(l)) on Scalar
                        ln_t = bscp.tile([1, 512], F32, tag="ln")
                        nc.scalar.activation(ln_t[:], l_ps[:], LNF)
                        r_sb = bscp.tile([1, 512], F32, tag="r")
                        nc.scalar.activation(r_sb[:], ln_t[:], EXPF,
                                             scale=-1.0)
                        rb = bscp.tile([128, 512], F32, tag="rb")
                        nc.gpsimd.partition_broadcast(rb[:], r_sb[:])
                        oo = oop.tile([128, 512], BF16, tag="oo")
                        nc.vector.tensor_mul(oo[:], o_ps[:], rb[:])
                        oo_tiles.append(oo)

                    # ---- C for this ib: y^T[:, isl] ----
                    for ct in range(16):
                        psc = pscp.tile([128, 512], F32, tag="psc")
                        for ht in range(8):
                            nc.tensor.matmul(
                                psc[:],
                                wps[:, ht, ct * 128:(ct + 1) * 128],
                                oo_tiles[ht][:], start=(ht == 0),
                                stop=(ht == 7))
                        yo = yop.tile([128, 512], F32, tag="yo")
                        nc.vector.tensor_copy(yo[:], psc[:])
                        nc.sync.dma_start(
                            yT[ct * 128:(ct + 1) * 128, isl], yo[:])

    nc.compile()
    return nc


def _prep_inputs(x, freqs_cos, freqs_sin, W_attn, b_attn, W_proj):
    """Host-side sharding / layout prep.  Returns list of 8 in_maps."""
    bf16 = mybir.dt.np(BF16)
    perm = np.concatenate([np.arange(0, HD, 2), np.arange(1, HD, 2)])

    cosT = np.ascontiguousarray(freqs_cos.T)  # [64, T]
    sinT = np.ascontiguousarray(freqs_sin.T)
    rtab_u = np.concatenate([cosT, cosT], axis=0).astype(bf16)
    rtab_v = np.concatenate([-sinT, sinT], axis=0).astype(bf16)
    cos4 = np.tile(freqs_cos, (1, 4)).astype(bf16)  # [T, 256]
    sin4 = np.tile(freqs_sin, (1, 4)).astype(bf16)

    jj = np.arange(128)[:, None]
    ii = np.arange(512)[None, :]
    masks = np.concatenate(
        [((r * 128 + jj) <= ii).astype(np.float32) for r in range(4)],
        axis=1).astype(bf16)  # [128, 2048]
    ones_in = np.ones((128, 1), dtype=bf16)

    in_maps = []
    for core in range(NCORES):
        b = core // 2
        hs = HC * (core % 2)
        cols = np.concatenate(
            [g * HD + perm for g in range(hs, hs + HC)])  # [1024]

        wqk = np.concatenate(
            [W_attn[:, cols], W_attn[:, C + cols]], axis=1)
        bqk_flat = np.concatenate([b_attn[cols], b_attn[C + cols]])
        bqk = np.ascontiguousarray(
            bqk_flat.reshape(16, 128).T)  # [128, 16], bias[jt*128+p]
        wv = W_attn[:, 2 * C + cols]
        bv = np.broadcast_to(b_attn[2 * C + cols], (128, 1024))
        wp = W_proj[cols, :]

        in_maps.append({
            "xT": np.ascontiguousarray(x[b].T).astype(bf16),
            "wqk": np.ascontiguousarray(wqk).astype(bf16),
            "bqk": np.ascontiguousarray(bqk).astype(np.float32),
            "wv": np.ascontiguousarray(wv).astype(bf16),
            "bv": np.ascontiguousarray(bv).astype(np.float32),
            "wp": np.ascontiguousarray(wp).astype(bf16),
            "rtab_u": rtab_u,
            "rtab_v": rtab_v,
            "cos4": cos4,
            "sin4": sin4,
            "masks": np.ascontiguousarray(masks),
            "ones_in": ones_in,
        })
    return in_maps


def kernel(x, freqs_cos, freqs_sin, mask, W_attn, b_attn, W_proj, b_proj,
           _return_results=False, _trace=False):
    x = np.asarray(x, dtype=np.float32)
    freqs_cos = np.asarray(freqs_cos, dtype=np.float32)
    freqs_sin = np.asarray(freqs_sin, dtype=np.float32)
    W_attn = np.asarray(W_attn, dtype=np.float32)
    b_attn = np.asarray(b_attn, dtype=np.float32)
    W_proj = np.asarray(W_proj, dtype=np.float32)
    b_proj = np.asarray(b_proj, dtype=np.float32)

    if "nc" not in _CACHE:
        _CACHE["nc"] = _build_nc()
    nc = _CACHE["nc"]

    in_maps = _prep_inputs(x, freqs_cos, freqs_sin, W_attn, b_attn, W_proj)
    res = run_bass_kernel_spmd(nc, in_maps, core_ids=list(range(NCORES)),
                               trace=_trace)

    out = np.empty((B, T, C), dtype=np.float32)
    for b in range(B):
        yt0 = res.results[2 * b]["yT"]
        yt1 = res.results[2 * b + 1]["yT"]
        out[b] = yt0.T + yt1.T + b_proj[None, :]
    if _return_results:
        return out, res
    return out
